# revision 15
# baseline (speedup 1.0000x reference)
"""Self-contained Trainium2 kernel for nn_ContextualizedNN (gnn_message_passing).

kernel(**inputs) takes the FULL unsharded inputs (as produced by the problem's
setup_inputs) and returns the full [8192] float32 output.

Strategy: data-parallel over the batch across 8 NeuronCores. Host prep (cheap
index plumbing, off the HW critical path): per core and per quarter (256
elements), dedupe the 25,600 neighbor draws so row ranks fit int16, and build
a 512-byte staging row per unique neighbor: [scr bf16 100 | pad | emb bf16 64
| pad]. Device per core (B_CORE=1024, tiles of TB=128):

  S^T:  ONE transpose-mode dma_gather per (tile, side): 12,928 int16 ranks
        pull the 256B scr half-rows and the 16-bit xbar writes them
        column-per-index -> sTg[j-part, (e,k)] = S_e^T directly. No PE
        transposes, no PSUM round-trip.
  E:    ONE dma_gather per (tile, side): 16,384 ranks (slot-padded to 128)
        land row i at partition i%128 -> gathE[j-part, e-block, 128].
  mm1:  scored_e = S_e^T.T @ E_e (bf16, 128-col FWL weight loads, fp32 PSUM,
        8 elems/bank); DVE copies each bank to tprime[k, (e,d)] bf16.
  mm2:  h[:, e] += W1[(k|side,d), :].T @ tprime[k, d-strided] over 128
        accumulating matmuls; relu(+b1); o = W2.T @ r; relu(+b2); sigmoid.
"""
import os
os.environ.setdefault("JAX_PLATFORMS", "cpu")
from contextlib import ExitStack

import numpy as np
import ml_dtypes

import concourse.bass as bass
import concourse.bacc as bacc
import concourse.tile as tile
from concourse import mybir
from concourse.bass_utils import run_bass_kernel_spmd

P = 128
K = 100
D = 64
HID = 128
N_USERS = 100000
N_ITEMS = 50000
B = 8192
N_CORES = 8
B_CORE = B // N_CORES
TB = 128            # batch tile (mm2 group)
Q = 256             # dedup scope (elements) so unique ranks fit int16
CAP = Q * K         # staging rows per quarter-side (upper bound on unique)
SROW = 256          # staging row: 256 bf16 = 512B: [scr 100|0pad|emb 64|0pad]
GE = 8              # elements per gather group
GC = 896            # idx per gather group (8*100 + 28 rank-0 + 68 x -1)
GV = 828            # valid (non-negative) idx per group
NG = TB // GE       # gather groups per tile-side (16)

F32 = mybir.dt.float32
BF16 = mybir.dt.bfloat16
I16 = mybir.dt.int16

NP_BF16 = ml_dtypes.bfloat16


def _build(b_core, tb):
    nc = bacc.Bacc("TRN2", target_bir_lowering=False, debug=False,
                   dynamic_dma_scratch_size=16384, num_swdge_queues=4)
    n_tiles = b_core // tb
    nq = b_core // Q

    stag_u = nc.dram_tensor("stag_u", [nq * CAP, SROW], BF16,
                            kind="ExternalInput").ap()
    stag_i = nc.dram_tensor("stag_i", [nq * CAP, SROW], BF16,
                            kind="ExternalInput").ap()
    sidx_u = nc.dram_tensor("sidx_u", [P, n_tiles * NG * (GC // 16)], I16,
                            kind="ExternalInput").ap()
    sidx_i = nc.dram_tensor("sidx_i", [P, n_tiles * NG * (GC // 16)], I16,
                            kind="ExternalInput").ap()
    w1p = nc.dram_tensor("w1p", [K, 2 * D * HID], BF16, kind="ExternalInput").ap()
    b1p = nc.dram_tensor("b1p", [HID, 1], F32, kind="ExternalInput").ap()
    w2p = nc.dram_tensor("w2p", [HID, 1], BF16, kind="ExternalInput").ap()
    b2p = nc.dram_tensor("b2p", [1, 1], F32, kind="ExternalInput").ap()
    out = nc.dram_tensor("out", [1, b_core], F32, kind="ExternalOutput").ap()

    SC = NG * (GC // 16)   # idx cols per tile-side (896)

    with tile.TileContext(nc) as tc:
        ctx = ExitStack()
        consts = ctx.enter_context(tc.tile_pool(name="consts", bufs=1))
        stp = ctx.enter_context(tc.tile_pool(name="stp", bufs=24))
        gep = ctx.enter_context(tc.tile_pool(name="gep", bufs=3))
        tpp = ctx.enter_context(tc.tile_pool(name="tpp", bufs=2))
        outp = ctx.enter_context(tc.tile_pool(name="outp", bufs=2))
        pst = ctx.enter_context(tc.tile_pool(name="pst", bufs=2, space="PSUM"))
        psm = ctx.enter_context(tc.tile_pool(name="psm", bufs=4, space="PSUM"))
        psh = ctx.enter_context(tc.tile_pool(name="psh", bufs=2, space="PSUM"))

        w1sb = consts.tile([P, 2 * D * HID], BF16)
        nc.sync.dma_start(out=w1sb[:K, :], in_=w1p[:, :])
        b1sb = consts.tile([P, 1], F32)
        nc.sync.dma_start(out=b1sb[:HID, :], in_=b1p[:, :])
        w2sb = consts.tile([P, 1], BF16)
        nc.sync.dma_start(out=w2sb[:HID, :], in_=w2p[:, :])
        b2sb = consts.tile([P, 1], F32)
        nc.sync.dma_start(out=b2sb[:1, :], in_=b2p[:, :])
        out_sb = consts.tile([1, b_core], F32)

        ident = consts.tile([P, P], BF16)
        from concourse.masks import make_identity
        make_identity(nc, ident[:])
        sxu = consts.tile([P, n_tiles * SC], I16)
        nc.sync.dma_start(out=sxu[:, :], in_=sidx_u[:, :])
        sxi = consts.tile([P, n_tiles * SC], I16)
        nc.sync.dma_start(out=sxi[:, :], in_=sidx_i[:, :])
        sides = [dict(stag=stag_u, sidx=sxu),
                 dict(stag=stag_i, sidx=sxi)]
        qrot = [0]

        for t in range(n_tiles):
            qoff = (t // (Q // tb)) * CAP
            tprimes = []
            for sd in sides:
                sidx = sd["sidx"][:, t * SC:(t + 1) * SC]

                tprime = tpp.tile([P, tb * D], BF16, tag="tp")
                tprimes.append(tprime)
                gs = [None] * NG
                esbs = [None] * NG

                def do_gather(gr):
                    g8 = stp.tile([P, 2 * GC], BF16, tag="g8")
                    nc.gpsimd.dma_gather(
                        out_ap=g8[:P, :].rearrange("p (o c) -> p o c", o=2),
                        in_ap=sd["stag"][qoff:qoff + CAP, 0:SROW],
                        idxs_ap=sidx[:, gr * (GC // 16):(gr + 1) * (GC // 16)],
                        num_idxs=GC,
                        num_idxs_reg=GV,
                        elem_size=SROW,
                        elem_step=SROW,
                        transpose=True,
                        single_packet=True,
                        queue_num=qrot[0] % 4,
                    )
                    qrot[0] += 1
                    gs[gr] = g8

                def do_etr(gr):
                    gv8 = gs[gr][:P, :].rearrange("p (o c) -> p o c", o=2)
                    et_ps = pst.tile([P, GE * D], BF16, space="PSUM", tag="et")
                    for q in range(GE):
                        nc.tensor.transpose(
                            out=et_ps[:P, q * D:(q + 1) * D],
                            in_=gv8[:D, 1, q * K:q * K + P],
                            identity=ident[:D, :D],
                        )
                    esb = gep.tile([P, GE * D], BF16, tag="esb")
                    nc.vector.tensor_copy(out=esb[:P, :], in_=et_ps[:P, :])
                    esbs[gr] = esb

                def do_mm1(gr):
                    gv8 = gs[gr][:P, :].rearrange("p (o c) -> p o c", o=2)
                    esb = esbs[gr]
                    sc_ps = psm.tile([P, 8 * D], F32, space="PSUM", tag="sc")
                    for q in range(GE):
                        nc.tensor.matmul(
                            out=sc_ps[:P, q * D:(q + 1) * D],
                            lhsT=gv8[:K, 0, q * K:q * K + P],
                            rhs=esb[:K, q * D:(q + 1) * D],
                            start=True, stop=True,
                        )
                    e0 = t_e0[0] + gr * GE
                    nc.vector.tensor_copy(
                        out=tprime[:K, gr * GE * D:(gr + 1) * GE * D],
                        in_=sc_ps[:K, :8 * D],
                    )

                t_e0 = [0]
                for gr in range(NG):
                    do_gather(gr)
                do_etr(0)
                for gr in range(1, NG):
                    do_etr(gr)
                    do_mm1(gr - 1)
                do_mm1(NG - 1)

            h_ps = psh.tile([P, tb], F32, space="PSUM", tag="h_ps")
            nmm = 2 * D
            m = 0
            for si, tprime in enumerate(tprimes):
                tpv = tprime[:K, :].rearrange("p (e d) -> p d e", d=D)
                for d in range(D):
                    nc.tensor.matmul(
                        out=h_ps[:HID, :tb],
                        lhsT=w1sb[:K, (si * D + d) * HID:(si * D + d + 1) * HID],
                        rhs=tpv[:, d:d + 1, :],
                        start=(m == 0), stop=(m == nmm - 1),
                    )
                    m += 1
            r_sb = outp.tile([P, tb], BF16, tag="r_sb")
            nc.scalar.activation(
                out=r_sb[:HID, :tb], in_=h_ps[:HID, :tb],
                func=mybir.ActivationFunctionType.Relu,
                bias=b1sb[:HID, :1], scale=1.0,
            )
            o_ps = psh.tile([P, tb], F32, space="PSUM", tag="h_ps")
            nc.tensor.matmul(
                out=o_ps[:1, :tb], lhsT=w2sb[:HID, :1], rhs=r_sb[:HID, :tb],
                start=True, stop=True,
            )
            o1 = outp.tile([P, tb], F32, tag="o1")
            nc.scalar.activation(
                out=o1[:1, :tb], in_=o_ps[:1, :tb],
                func=mybir.ActivationFunctionType.Relu,
                bias=b2sb[:1, :1], scale=1.0,
            )
            nc.scalar.activation(
                out=out_sb[:1, t * tb:(t + 1) * tb], in_=o1[:1, :tb],
                func=mybir.ActivationFunctionType.Sigmoid,
            )
        nc.sync.dma_start(out=out[:1, :], in_=out_sb[:1, :])
        ctx.close()

    nc.compile()
    return nc


_NC_CACHE = {}


def _get_nc():
    key = (B_CORE, TB)
    if key not in _NC_CACHE:
        _NC_CACHE[key] = _build(B_CORE, TB)
    return _NC_CACHE[key]


def _pack_weights(W1, b1, W2, b2):
    w1p = np.ascontiguousarray(
        np.asarray(W1, np.float32)
        .reshape(2, K, D, HID).transpose(1, 0, 2, 3).reshape(K, 2 * D * HID)
        .astype(NP_BF16)
    )
    w2p = np.ascontiguousarray(
        np.asarray(W2, np.float32).reshape(HID, 1).astype(NP_BF16)
    )
    b1p = np.ascontiguousarray(np.asarray(b1, np.float32).reshape(HID, 1))
    b2p = np.ascontiguousarray(np.asarray(b2, np.float32).reshape(1, 1))
    return w1p, b1p, w2p, b2p


def _wrap16(flat):
    """int16 flat index list -> [16, len/16] wrapped, tiled to 128 rows."""
    w = flat.reshape(-1, 16).T.astype(np.int16)        # [16, n/16]
    return np.ascontiguousarray(np.tile(w, (8, 1)))     # [128, n/16]


def _prep_side(neigh, scr, emb, b_core):
    """neigh: [b_core, K] int64/32 for one core+side.

    Returns stag [nq*CAP, SROW] bf16, sidx [128, n_tiles*SC] i16,
    eidx [128, n_tiles*EC] i16.
    """
    n_tiles = b_core // TB
    nq = b_core // Q
    scr = np.asarray(scr, np.float32)
    emb = np.asarray(emb, np.float32)
    stag = np.zeros((nq * CAP, SROW), dtype=NP_BF16)
    sidx_cols = []
    for q in range(nq):
        draws = np.asarray(neigh[q * Q:(q + 1) * Q], np.int64)  # [Q, K]
        uniq, inv = np.unique(draws, return_inverse=True)
        inv = inv.reshape(Q, K).astype(np.int16)                # ranks
        u = len(uniq)
        stag[q * CAP:q * CAP + u, 0:K] = scr[uniq].astype(NP_BF16)
        stag[q * CAP:q * CAP + u, P:P + D] = emb[uniq].astype(NP_BF16)
        for tq in range(Q // TB):
            r = inv[tq * TB:(tq + 1) * TB]                      # [TB, K]
            for g in range(NG):
                rg = r[g * GE:(g + 1) * GE].reshape(-1)         # 800
                sflat = np.concatenate([
                    rg, np.zeros(28, np.int16),
                    np.full(68, -1, np.int16)])                 # GC=896
                sidx_cols.append(_wrap16(sflat))
    sidx = np.concatenate(sidx_cols, axis=1)
    assert sidx.shape == (P, n_tiles * NG * (GC // 16))
    return stag, np.ascontiguousarray(sidx)


def kernel(user_idxs, item_idxs, user_idx_tensor, item_idx_tensor,
           user_scr_tensor, item_scr_tensor, user_emb, item_emb,
           W1, b1, W2, b2, _trace=False):
    nc = _get_nc()
    w1p, b1p, w2p, b2p = _pack_weights(W1, b1, W2, b2)

    nu = np.asarray(user_idx_tensor, np.int64)[np.asarray(user_idxs)]
    ni = np.asarray(item_idx_tensor, np.int64)[np.asarray(item_idxs)]

    common = dict(w1p=w1p, b1p=b1p, w2p=w2p, b2p=b2p)
    in_maps = []
    for c in range(N_CORES):
        m = dict(common)
        su, xu = _prep_side(nu[c * B_CORE:(c + 1) * B_CORE],
                            user_scr_tensor, user_emb, B_CORE)
        si_, xi = _prep_side(ni[c * B_CORE:(c + 1) * B_CORE],
                             item_scr_tensor, item_emb, B_CORE)
        m.update(stag_u=su, sidx_u=xu, stag_i=si_, sidx_i=xi)
        in_maps.append(m)

    res = run_bass_kernel_spmd(nc, in_maps, list(range(N_CORES)), trace=_trace)
    out = np.concatenate([res.results[c]["out"][0] for c in range(N_CORES)])
    if _trace:
        kernel._last_exec_time_ns = res.exec_time_ns
        kernel._last_results = res
    return out


# revision 16
# speedup vs baseline: 1.0046x; 1.0046x over previous
"""Self-contained Trainium2 kernel for nn_ContextualizedNN (gnn_message_passing).

kernel(**inputs) takes the FULL unsharded inputs (as produced by the problem's
setup_inputs) and returns the full [8192] float32 output.

Strategy: data-parallel over the batch across 8 NeuronCores. Host prep (cheap
index plumbing, off the HW critical path): per core and per quarter (256
elements), dedupe the 25,600 neighbor draws so row ranks fit int16, and build
a 512-byte staging row per unique neighbor: [scr bf16 100 | pad | emb bf16 64
| pad]. Device per core (B_CORE=1024, tiles of TB=128):

  gather: per (tile, side), 16 transpose-mode dma_gathers of 896 int16 ranks
        (8 elements x 100 slots + 28 rank-0 + 68 trailing -1), one 512B
        staging row per reference, round-robin over 4 SWDGE queues (the Q7
        descriptor loop is ~10ns/idx per queue; 4 queues run concurrently).
        The 16-bit xbar transpose writes each row column-per-index, yielding
        plane 0 = S_e^T[j-part, k-cols] directly and plane 1 = E_e^T.
  E:    plane 1 is PE-transposed per element ([64,128] -> [128,64], FWL
        weight loads), DVE drains each 8-element PSUM bank to SBUF.
  mm1:  scored_e = S_e^T.T @ E_e (bf16, 128-col FWL loads, fp32 PSUM,
        8 elems/bank); DVE copies each bank to tprime[k, (e,d)] bf16.
  mm2:  h[:, e] += W1[(k|side,d), :].T @ tprime[k, d-strided] over 128
        accumulating matmuls; relu(+b1); o = W2.T @ r; relu(+b2); sigmoid.
"""
import os
os.environ.setdefault("JAX_PLATFORMS", "cpu")
from contextlib import ExitStack

import numpy as np
import ml_dtypes

import concourse.bass as bass
import concourse.bacc as bacc
import concourse.tile as tile
from concourse import mybir
from concourse.bass_utils import run_bass_kernel_spmd

P = 128
K = 100
D = 64
HID = 128
N_USERS = 100000
N_ITEMS = 50000
B = 8192
N_CORES = 8
B_CORE = B // N_CORES
TB = 128            # batch tile (mm2 group)
Q = 256             # dedup scope (elements) so unique ranks fit int16
CAP = Q * K         # staging rows per quarter-side (upper bound on unique)
SROW = 256          # staging row: 256 bf16 = 512B: [scr 100|0pad|emb 64|0pad]
GE = 8              # elements per gather group
GC = 896            # idx per gather group (8*100 + 28 rank-0 + 68 x -1)
GV = 828            # valid (non-negative) idx per group
NG = TB // GE       # gather groups per tile-side (16)

F32 = mybir.dt.float32
BF16 = mybir.dt.bfloat16
I16 = mybir.dt.int16

NP_BF16 = ml_dtypes.bfloat16


def _build(b_core, tb):
    nc = bacc.Bacc("TRN2", target_bir_lowering=False, debug=False,
                   dynamic_dma_scratch_size=16384, num_swdge_queues=4)
    n_tiles = b_core // tb
    nq = b_core // Q

    stag_u = nc.dram_tensor("stag_u", [nq * CAP, SROW], BF16,
                            kind="ExternalInput").ap()
    stag_i = nc.dram_tensor("stag_i", [nq * CAP, SROW], BF16,
                            kind="ExternalInput").ap()
    sidx_u = nc.dram_tensor("sidx_u", [P, n_tiles * NG * (GC // 16)], I16,
                            kind="ExternalInput").ap()
    sidx_i = nc.dram_tensor("sidx_i", [P, n_tiles * NG * (GC // 16)], I16,
                            kind="ExternalInput").ap()
    w1p = nc.dram_tensor("w1p", [K, 2 * D * HID], BF16, kind="ExternalInput").ap()
    b1p = nc.dram_tensor("b1p", [HID, 1], F32, kind="ExternalInput").ap()
    w2p = nc.dram_tensor("w2p", [HID, 1], BF16, kind="ExternalInput").ap()
    b2p = nc.dram_tensor("b2p", [1, 1], F32, kind="ExternalInput").ap()
    out = nc.dram_tensor("out", [1, b_core], F32, kind="ExternalOutput").ap()

    SC = NG * (GC // 16)   # idx cols per tile-side (896)

    with tile.TileContext(nc) as tc:
        ctx = ExitStack()
        consts = ctx.enter_context(tc.tile_pool(name="consts", bufs=1))
        stp = ctx.enter_context(tc.tile_pool(name="stp", bufs=24))
        gep = ctx.enter_context(tc.tile_pool(name="gep", bufs=3))
        tpp = ctx.enter_context(tc.tile_pool(name="tpp", bufs=2))
        outp = ctx.enter_context(tc.tile_pool(name="outp", bufs=2))
        pst = ctx.enter_context(tc.tile_pool(name="pst", bufs=2, space="PSUM"))
        psm = ctx.enter_context(tc.tile_pool(name="psm", bufs=4, space="PSUM"))
        psh = ctx.enter_context(tc.tile_pool(name="psh", bufs=2, space="PSUM"))

        w1sb = consts.tile([P, 2 * D * HID], BF16)
        nc.sync.dma_start(out=w1sb[:K, :], in_=w1p[:, :])
        b1sb = consts.tile([P, 1], F32)
        nc.sync.dma_start(out=b1sb[:HID, :], in_=b1p[:, :])
        w2sb = consts.tile([P, 1], BF16)
        nc.sync.dma_start(out=w2sb[:HID, :], in_=w2p[:, :])
        b2sb = consts.tile([P, 1], F32)
        nc.sync.dma_start(out=b2sb[:1, :], in_=b2p[:, :])
        out_sb = consts.tile([1, b_core], F32)

        ident = consts.tile([P, P], BF16)
        from concourse.masks import make_identity
        make_identity(nc, ident[:])
        sxu = consts.tile([P, n_tiles * SC], I16)
        nc.sync.dma_start(out=sxu[:, :], in_=sidx_u[:, :])
        sxi = consts.tile([P, n_tiles * SC], I16)
        nc.sync.dma_start(out=sxi[:, :], in_=sidx_i[:, :])
        sides = [dict(stag=stag_u, sidx=sxu),
                 dict(stag=stag_i, sidx=sxi)]
        qrot = [0]

        for t in range(n_tiles):
            qoff = (t // (Q // tb)) * CAP
            tprimes = []
            for sd in sides:
                sidx = sd["sidx"][:, t * SC:(t + 1) * SC]

                tprime = tpp.tile([P, tb * D], BF16, tag="tp")
                tprimes.append(tprime)
                gs = [None] * NG
                esbs = [None] * NG

                def do_gather(gr):
                    g8 = stp.tile([P, 2 * GC], BF16, tag="g8")
                    nc.gpsimd.dma_gather(
                        out_ap=g8[:P, :].rearrange("p (o c) -> p o c", o=2),
                        in_ap=sd["stag"][qoff:qoff + CAP, 0:SROW],
                        idxs_ap=sidx[:, gr * (GC // 16):(gr + 1) * (GC // 16)],
                        num_idxs=GC,
                        num_idxs_reg=GV,
                        elem_size=SROW,
                        elem_step=SROW,
                        transpose=True,
                        single_packet=True,
                        queue_num=qrot[0] % 4,
                    )
                    qrot[0] += 1
                    gs[gr] = g8

                def do_etr(gr):
                    gv8 = gs[gr][:P, :].rearrange("p (o c) -> p o c", o=2)
                    et_ps = pst.tile([P, GE * D], BF16, space="PSUM", tag="et")
                    for q in range(GE):
                        nc.tensor.transpose(
                            out=et_ps[:P, q * D:(q + 1) * D],
                            in_=gv8[:D, 1, q * K:q * K + P],
                            identity=ident[:D, :D],
                        )
                    esb = gep.tile([P, GE * D], BF16, tag="esb")
                    nc.vector.tensor_copy(out=esb[:P, :], in_=et_ps[:P, :])
                    esbs[gr] = esb

                def do_mm1(gr):
                    gv8 = gs[gr][:P, :].rearrange("p (o c) -> p o c", o=2)
                    esb = esbs[gr]
                    sc_ps = psm.tile([P, 8 * D], F32, space="PSUM", tag="sc")
                    for q in range(GE):
                        nc.tensor.matmul(
                            out=sc_ps[:P, q * D:(q + 1) * D],
                            lhsT=gv8[:K, 0, q * K:q * K + P],
                            rhs=esb[:K, q * D:(q + 1) * D],
                            start=True, stop=True,
                        )
                    nc.vector.tensor_copy(
                        out=tprime[:K, gr * GE * D:(gr + 1) * GE * D],
                        in_=sc_ps[:K, :8 * D],
                    )

                for gr in range(NG):
                    do_gather(gr)
                do_etr(0)
                for gr in range(1, NG):
                    do_etr(gr)
                    do_mm1(gr - 1)
                do_mm1(NG - 1)

            h_ps = psh.tile([P, tb], F32, space="PSUM", tag="h_ps")
            nmm = 2 * D
            m = 0
            for si, tprime in enumerate(tprimes):
                tpv = tprime[:K, :].rearrange("p (e d) -> p d e", d=D)
                for d in range(D):
                    nc.tensor.matmul(
                        out=h_ps[:HID, :tb],
                        lhsT=w1sb[:K, (si * D + d) * HID:(si * D + d + 1) * HID],
                        rhs=tpv[:, d:d + 1, :],
                        start=(m == 0), stop=(m == nmm - 1),
                    )
                    m += 1
            r_sb = outp.tile([P, tb], BF16, tag="r_sb")
            nc.scalar.activation(
                out=r_sb[:HID, :tb], in_=h_ps[:HID, :tb],
                func=mybir.ActivationFunctionType.Relu,
                bias=b1sb[:HID, :1], scale=1.0,
            )
            o_ps = psh.tile([P, tb], F32, space="PSUM", tag="h_ps")
            nc.tensor.matmul(
                out=o_ps[:1, :tb], lhsT=w2sb[:HID, :1], rhs=r_sb[:HID, :tb],
                start=True, stop=True,
            )
            o1 = outp.tile([P, tb], F32, tag="o1")
            nc.scalar.activation(
                out=o1[:1, :tb], in_=o_ps[:1, :tb],
                func=mybir.ActivationFunctionType.Relu,
                bias=b2sb[:1, :1], scale=1.0,
            )
            nc.scalar.activation(
                out=out_sb[:1, t * tb:(t + 1) * tb], in_=o1[:1, :tb],
                func=mybir.ActivationFunctionType.Sigmoid,
            )
        nc.sync.dma_start(out=out[:1, :], in_=out_sb[:1, :])
        ctx.close()

    nc.compile()
    return nc


_NC_CACHE = {}


def _get_nc():
    key = (B_CORE, TB)
    if key not in _NC_CACHE:
        _NC_CACHE[key] = _build(B_CORE, TB)
    return _NC_CACHE[key]


def _pack_weights(W1, b1, W2, b2):
    w1p = np.ascontiguousarray(
        np.asarray(W1, np.float32)
        .reshape(2, K, D, HID).transpose(1, 0, 2, 3).reshape(K, 2 * D * HID)
        .astype(NP_BF16)
    )
    w2p = np.ascontiguousarray(
        np.asarray(W2, np.float32).reshape(HID, 1).astype(NP_BF16)
    )
    b1p = np.ascontiguousarray(np.asarray(b1, np.float32).reshape(HID, 1))
    b2p = np.ascontiguousarray(np.asarray(b2, np.float32).reshape(1, 1))
    return w1p, b1p, w2p, b2p


def _wrap16(flat):
    """int16 flat index list -> [16, len/16] wrapped, tiled to 128 rows."""
    w = flat.reshape(-1, 16).T.astype(np.int16)        # [16, n/16]
    return np.ascontiguousarray(np.tile(w, (8, 1)))     # [128, n/16]


def _prep_side(neigh, scr, emb, b_core):
    """neigh: [b_core, K] int64/32 for one core+side.

    Returns stag [nq*CAP, SROW] bf16, sidx [128, n_tiles*SC] i16,
    eidx [128, n_tiles*EC] i16.
    """
    n_tiles = b_core // TB
    nq = b_core // Q
    scr = np.asarray(scr, np.float32)
    emb = np.asarray(emb, np.float32)
    stag = np.zeros((nq * CAP, SROW), dtype=NP_BF16)
    sidx_cols = []
    for q in range(nq):
        draws = np.asarray(neigh[q * Q:(q + 1) * Q], np.int64)  # [Q, K]
        uniq, inv = np.unique(draws, return_inverse=True)
        inv = inv.reshape(Q, K).astype(np.int16)                # ranks
        u = len(uniq)
        stag[q * CAP:q * CAP + u, 0:K] = scr[uniq].astype(NP_BF16)
        stag[q * CAP:q * CAP + u, P:P + D] = emb[uniq].astype(NP_BF16)
        for tq in range(Q // TB):
            r = inv[tq * TB:(tq + 1) * TB]                      # [TB, K]
            for g in range(NG):
                rg = r[g * GE:(g + 1) * GE].reshape(-1)         # 800
                sflat = np.concatenate([
                    rg, np.zeros(28, np.int16),
                    np.full(68, -1, np.int16)])                 # GC=896
                sidx_cols.append(_wrap16(sflat))
    sidx = np.concatenate(sidx_cols, axis=1)
    assert sidx.shape == (P, n_tiles * NG * (GC // 16))
    return stag, np.ascontiguousarray(sidx)


def kernel(user_idxs, item_idxs, user_idx_tensor, item_idx_tensor,
           user_scr_tensor, item_scr_tensor, user_emb, item_emb,
           W1, b1, W2, b2, _trace=False):
    nc = _get_nc()
    w1p, b1p, w2p, b2p = _pack_weights(W1, b1, W2, b2)

    nu = np.asarray(user_idx_tensor, np.int64)[np.asarray(user_idxs)]
    ni = np.asarray(item_idx_tensor, np.int64)[np.asarray(item_idxs)]

    common = dict(w1p=w1p, b1p=b1p, w2p=w2p, b2p=b2p)
    in_maps = []
    for c in range(N_CORES):
        m = dict(common)
        su, xu = _prep_side(nu[c * B_CORE:(c + 1) * B_CORE],
                            user_scr_tensor, user_emb, B_CORE)
        si_, xi = _prep_side(ni[c * B_CORE:(c + 1) * B_CORE],
                             item_scr_tensor, item_emb, B_CORE)
        m.update(stag_u=su, sidx_u=xu, stag_i=si_, sidx_i=xi)
        in_maps.append(m)

    res = run_bass_kernel_spmd(nc, in_maps, list(range(N_CORES)), trace=_trace)
    out = np.concatenate([res.results[c]["out"][0] for c in range(N_CORES)])
    if _trace:
        kernel._last_exec_time_ns = res.exec_time_ns
        kernel._last_results = res
    return out
